# revision 17
# baseline (speedup 1.0000x reference)
"""Trainium2 Bass kernel for 16-head self-attention (B=1, T=2048, d=1024).

Sharding: 2 heads per NeuronCore (tensor-parallel over QKV columns / proj
rows) across 8 cores; each core emits a partial [T, d] projection output,
summed on the host.

Device-side layout (per core, heads h0/h1 stacked on SBUF partitions):
  qT/kT  [128, T]   head-dim-major (h0 dims on partitions 0-63, h1 on 64-127)
  scores ST[k, q] via row-tiled matmul pairs (both heads concurrent on PE)
  exp on ACT (no max subtraction needed: logits are O(1) by construction)
  y^T = [v | 1]^T @ exp(ST)  -> row 64 of the PSUM tile is the softmax sum
  per-head proj + per-partition renormalization (sums transposed via DMA)
"""
import sys, os, types

sys.path.insert(0, "/opt/trn_rl_repo")

import numpy as np
import ml_dtypes

import concourse.bass as bass
import concourse.bacc as bacc
import concourse.tile as tile
from concourse import mybir
from concourse import bass_utils

BF16 = mybir.dt.bfloat16
F32 = mybir.dt.float32

B, T, D = 1, 2048, 1024
H = 16
HD = D // H          # 64
NCORES = 8
HPD = H // NCORES    # 2 heads per device
DL = HPD * HD        # 128 local head dims per device
QG = 4               # q groups of 512
QGS = T // QG        # 512
KT = T // 128        # 16 k tiles
NG = D // 512        # 2 output column groups

last_results = None  # BassKernelResults of the most recent run (for test.py)


DEBUG_DUMPS = os.environ.get("KERNEL_DEBUG_DUMPS", "0") == "1"


def build_nc():
    nc = bacc.Bacc("TRN2", target_bir_lowering=False, debug=False,
                   num_devices=NCORES)
    xT = nc.dram_tensor("xT", [D, T], BF16, kind="ExternalInput").ap()
    wq = nc.dram_tensor("wq", [D, DL], BF16, kind="ExternalInput").ap()
    wk = nc.dram_tensor("wk", [D, DL], BF16, kind="ExternalInput").ap()
    wv = nc.dram_tensor("wv", [D, DL], BF16, kind="ExternalInput").ap()
    wp = nc.dram_tensor("wp", [DL, D], BF16, kind="ExternalInput").ap()
    out = nc.dram_tensor("out", [T, D], F32, kind="ExternalOutput").ap()
    dbg = {}
    if DEBUG_DUMPS:
        for name, shape, dt in [
            ("dbg_qT", [128, T], BF16), ("dbg_kT", [128, T], BF16),
            ("dbg_v0", [128, KT * 65], BF16),
            ("dbg_v1", [128, KT * 65], BF16),
            ("dbg_e0", [128, QGS], BF16),
            ("dbg_yT", [64, HPD, T], BF16),
            ("dbg_spre", [128, HPD, 16], F32),
            ("dbg_scal", [128, HPD, 16], F32),
        ]:
            dbg[name] = nc.dram_tensor(name, shape, dt,
                                       kind="ExternalOutput").ap()

    with tile.TileContext(nc) as tc:
        with (
            tc.tile_pool(name="const", bufs=1) as cpool,
            tc.tile_pool(name="work", bufs=4) as wpool,
            tc.tile_pool(name="epool", bufs=4) as epool,
            tc.tile_pool(name="opool", bufs=4) as opool,
            tc.tile_pool(name="dram", bufs=1, space="DRAM") as dram_pool,
            tc.tile_pool(name="mm", bufs=2, space="PSUM") as mm_psum,
            tc.tile_pool(name="sc", bufs=2, space="PSUM") as sc_psum,
            tc.tile_pool(name="yp", bufs=2, space="PSUM") as y_psum_pool,
        ):
            # ---- persistent SBUF tensors ----
            xT_sb = cpool.tile([128, 8, T], BF16)       # x^T, d-major
            wq_sb = cpool.tile([128, 8, DL], BF16)
            wk_sb = cpool.tile([128, 8, DL], BF16)
            wv_sb = cpool.tile([128, 8, DL], BF16)
            wp0_sb = cpool.tile([64, D], BF16)          # proj rows of head 0
            wp1_sb = cpool.tile([64, D], BF16)          # proj rows of head 1
            qT_sb = cpool.tile([128, T], BF16)
            kT_sb = cpool.tile([128, T], BF16)
            v0_sb = cpool.tile([128, KT * 65], BF16)    # [t, v|1] head 0
            v1_sb = cpool.tile([128, KT * 65], BF16)
            yT_sb = cpool.tile([64, HPD, T], BF16)      # y^T per head
            spre_sb = cpool.tile([128, HPD, 16], F32)   # softmax sums (q-major)
            scal_sb = cpool.tile([128, HPD, 16], F32)   # 1/sums
            sums_dr = dram_pool.tile([HPD, QG, QGS], F32)  # DRAM bounce buffer

            # ---- input DMAs (split along d so matmuls can start early) ----
            xTr = xT.rearrange("(n p) t -> p n t", p=128)
            wqr = wq.rearrange("(n p) m -> p n m", p=128)
            wkr = wk.rearrange("(n p) m -> p n m", p=128)
            wvr = wv.rearrange("(n p) m -> p n m", p=128)
            for kk in range(8):
                nc.sync.dma_start(xT_sb[:, kk, :], xTr[:, kk, :])
                nc.sync.dma_start(wq_sb[:, kk, :], wqr[:, kk, :])
                nc.sync.dma_start(wk_sb[:, kk, :], wkr[:, kk, :])
                nc.sync.dma_start(wv_sb[:, kk, :], wvr[:, kk, :])
            nc.sync.dma_start(wp0_sb[:], wp[0:64, :])
            nc.sync.dma_start(wp1_sb[:], wp[64:128, :])

            nc.vector.memset(v0_sb[:], 1.0)
            nc.vector.memset(v1_sb[:], 1.0)

            # ---- phase 1: qT / kT = W^T @ x^T  ([128, T] each) ----
            for dst, w_sb in ((qT_sb, wq_sb), (kT_sb, wk_sb)):
                for g in range(QG):
                    ps = mm_psum.tile([128, QGS], F32, tag="mmps")
                    for kk in range(8):
                        nc.tensor.matmul(
                            ps[:], w_sb[:, kk, :],
                            xT_sb[:, kk, g * QGS:(g + 1) * QGS],
                            start=(kk == 0), stop=(kk == 7))
                    nc.vector.tensor_copy(dst[:, g * QGS:(g + 1) * QGS], ps[:])

            # v in natural [t, vdim] layout: lhsT = xT tile (t on out
            # partitions); both heads' halves land in the ones-padded bufs
            for tt in range(KT):
                ps = mm_psum.tile([128, DL], F32, tag="mmps")
                for kk in range(8):
                    nc.tensor.matmul(
                        ps[:], xT_sb[:, kk, tt * 128:(tt + 1) * 128],
                        wv_sb[:, kk, :], start=(kk == 0), stop=(kk == 7))
                nc.vector.tensor_copy(v0_sb[:, tt * 65: tt * 65 + 64],
                                      ps[:, 0:64])
                nc.vector.tensor_copy(v1_sb[:, tt * 65: tt * 65 + 64],
                                      ps[:, 64:128])

            # ---- phase 2: attention per q-group ----
            for g in range(QG):
                qsl = slice(g * QGS, (g + 1) * QGS)
                y0 = y_psum_pool.tile([65, QGS], F32, tag="y")
                y1 = y_psum_pool.tile([65, QGS], F32, tag="y")
                for kk in range(KT):
                    ksl = slice(kk * 128, (kk + 1) * 128)
                    # both heads' scores for this k-tile in ONE 2-bank tile:
                    # the row-tiled pair gates on the same release, stays
                    # adjacent in the PE stream, and overlaps (rows 0-63 vs
                    # 64-127 of the array)
                    sc = sc_psum.tile([128, 2 * QGS], F32, tag="sc")
                    nc.tensor.matmul(sc[:, 0:QGS], kT_sb[0:64, ksl],
                                     qT_sb[0:64, qsl], start=True,
                                     stop=True, tile_position=(0, 0))
                    nc.tensor.matmul(sc[:, QGS:2 * QGS], kT_sb[64:128, ksl],
                                     qT_sb[64:128, qsl], start=True,
                                     stop=True, tile_position=(64, 0))
                    # one exp covers both heads
                    e = epool.tile([128, 2 * QGS], BF16, tag="e")
                    nc.scalar.activation(e[:], sc[:],
                                         mybir.ActivationFunctionType.Exp)
                    if DEBUG_DUMPS and g == 0 and kk == 0:
                        nc.sync.dma_start(dbg["dbg_e0"][:], e[:, 0:QGS])
                    nc.tensor.matmul(y0[:], v0_sb[:, kk * 65: kk * 65 + 65],
                                     e[:, 0:QGS], start=(kk == 0),
                                     stop=(kk == KT - 1))
                    nc.tensor.matmul(y1[:], v1_sb[:, kk * 65: kk * 65 + 65],
                                     e[:, QGS:2 * QGS], start=(kk == 0),
                                     stop=(kk == KT - 1))
                # y rows 0-63 -> bf16 yT; row 64 (sums) -> fp32, transposed
                nc.vector.tensor_copy(yT_sb[:, 0, qsl], y0[0:64, :])
                nc.vector.tensor_copy(yT_sb[:, 1, qsl], y1[0:64, :])
                srow0 = wpool.tile([65, QGS], F32, tag="srow")
                srow1 = wpool.tile([65, QGS], F32, tag="srow")
                nc.vector.tensor_copy(srow0[64:65, :], y0[64:65, :])
                nc.vector.tensor_copy(srow1[64:65, :], y1[64:65, :])
                # transpose sums [1, 512] -> [128, 4] via a DRAM bounce
                # (SBUF APs can't move free elements onto partitions; DRAM
                # APs are linear so the scatter is legal on the read-back)
                nc.sync.dma_start(sums_dr[0, g, :], srow0[64:65, :])
                nc.sync.dma_start(sums_dr[1, g, :], srow1[64:65, :])
                for h in range(HPD):
                    nc.sync.dma_start(
                        spre_sb[:, h, g * 4:(g + 1) * 4],
                        sums_dr[h, g, :].rearrange("(c p) -> p c", c=4, p=128))
                nc.vector.reciprocal(scal_sb[:, :, g * 4:(g + 1) * 4],
                                     spre_sb[:, :, g * 4:(g + 1) * 4])

                # proj for this q-group (overlaps the next group's attention)
                for qt in range(g * 4, (g + 1) * 4):
                    tsl = slice(qt * 128, (qt + 1) * 128)
                    for ngi in range(NG):
                        nsl = slice(ngi * 512, (ngi + 1) * 512)
                        pj0 = mm_psum.tile([128, 512], F32, tag="mmps")
                        pj1 = mm_psum.tile([128, 512], F32, tag="mmps")
                        nc.tensor.matmul(pj0[:], yT_sb[:, 0, tsl],
                                         wp0_sb[:, nsl], start=True, stop=True)
                        nc.tensor.matmul(pj1[:], yT_sb[:, 1, tsl],
                                         wp1_sb[:, nsl], start=True, stop=True)
                        acc = opool.tile([128, 512], F32, tag="acc")
                        o = opool.tile([128, 512], F32, tag="o")
                        nc.vector.tensor_scalar_mul(
                            acc[:], pj0[:], scal_sb[:, 0, qt:qt + 1])
                        nc.vector.scalar_tensor_tensor(
                            o[:], pj1[:], scal_sb[:, 1, qt:qt + 1], acc[:],
                            op0=mybir.AluOpType.mult, op1=mybir.AluOpType.add)
                        nc.sync.dma_start(out[tsl, nsl], o[:])

            if DEBUG_DUMPS:
                nc.sync.dma_start(dbg["dbg_qT"][:], qT_sb[:])
                nc.sync.dma_start(dbg["dbg_kT"][:], kT_sb[:])
                nc.sync.dma_start(dbg["dbg_v0"][:], v0_sb[:])
                nc.sync.dma_start(dbg["dbg_v1"][:], v1_sb[:])
                nc.sync.dma_start(dbg["dbg_yT"][:], yT_sb[:])
                nc.sync.dma_start(dbg["dbg_spre"][:], spre_sb[:])
                nc.sync.dma_start(dbg["dbg_scal"][:], scal_sb[:])

    nc.compile()
    return nc


_nc_cache = None


def kernel(x: np.ndarray, W_qkv: np.ndarray, W_proj: np.ndarray) -> np.ndarray:
    global _nc_cache, last_results
    assert x.shape == (B, T, D)
    x2d = np.ascontiguousarray(x.reshape(T, D))
    xT = np.ascontiguousarray(x2d.T).astype(ml_dtypes.bfloat16)
    scale = 1.0 / np.sqrt(np.float32(HD))

    in_maps = []
    for dev in range(NCORES):
        cs = slice(dev * DL, (dev + 1) * DL)
        wq = (W_qkv[:, cs.start:cs.stop] * scale).astype(ml_dtypes.bfloat16)
        wk = W_qkv[:, D + dev * DL: D + (dev + 1) * DL].astype(ml_dtypes.bfloat16)
        wv = W_qkv[:, 2 * D + dev * DL: 2 * D + (dev + 1) * DL].astype(
            ml_dtypes.bfloat16)
        wp = W_proj[dev * DL:(dev + 1) * DL, :].astype(ml_dtypes.bfloat16)
        in_maps.append({
            "xT": xT,
            "wq": np.ascontiguousarray(wq),
            "wk": np.ascontiguousarray(wk),
            "wv": np.ascontiguousarray(wv),
            "wp": np.ascontiguousarray(wp),
        })

    if _nc_cache is None:
        _nc_cache = build_nc()
    res = bass_utils.run_bass_kernel_spmd(
        _nc_cache, in_maps, core_ids=list(range(NCORES)),
        tmpdir=os.environ.get("BASS_KERNEL_TMPDIR"))
    last_results = res
    total = np.zeros((T, D), dtype=np.float64)
    for dev in range(NCORES):
        total += res.results[dev]["out"].astype(np.float64)
    return total.astype(np.float32).reshape(B, T, D)


if __name__ == "__main__":
    rng = np.random.default_rng(0)
    x = rng.standard_normal((B, T, D)).astype(np.float32)
    wqkv = (rng.standard_normal((D, 3 * D)) * 0.02).astype(np.float32)
    wproj = (rng.standard_normal((D, D)) * 0.02).astype(np.float32)
    y = kernel(x, wqkv, wproj)
    print("kernel output", y.shape, y.dtype, float(np.abs(y).mean()))


# revision 18
# speedup vs baseline: 1.0272x; 1.0272x over previous
"""Trainium2 Bass kernel for 16-head self-attention (B=1, T=2048, d=1024).

Sharding: 2 heads per NeuronCore (tensor-parallel over QKV columns / proj
rows) across 8 cores; each core emits a partial [T, d] projection output,
summed on the host.

Device-side layout (per core, heads h0/h1 stacked on SBUF partitions):
  qT/kT  [128, T]   head-dim-major (h0 dims on partitions 0-63, h1 on 64-127)
  scores ST[k, q] via row-tiled matmul pairs (both heads concurrent on PE)
  exp on ACT (no max subtraction needed: logits are O(1) by construction)
  y^T = [v | 1]^T @ exp(ST)  -> row 64 of the PSUM tile is the softmax sum
  per-head proj + per-partition renormalization (sums transposed via DMA)
"""
import sys, os, types

sys.path.insert(0, "/opt/trn_rl_repo")

import numpy as np
import ml_dtypes

import concourse.bass as bass
import concourse.bacc as bacc
import concourse.tile as tile
from concourse import mybir
from concourse import bass_utils

BF16 = mybir.dt.bfloat16
F32 = mybir.dt.float32

B, T, D = 1, 2048, 1024
H = 16
HD = D // H          # 64
NCORES = 8
HPD = H // NCORES    # 2 heads per device
DL = HPD * HD        # 128 local head dims per device
QG = 4               # q groups of 512
QGS = T // QG        # 512
KT = T // 128        # 16 k tiles
NG = D // 512        # 2 output column groups

last_results = None  # BassKernelResults of the most recent run (for test.py)


DEBUG_DUMPS = os.environ.get("KERNEL_DEBUG_DUMPS", "0") == "1"


def build_nc():
    nc = bacc.Bacc("TRN2", target_bir_lowering=False, debug=False,
                   num_devices=NCORES)
    xT = nc.dram_tensor("xT", [D, T], BF16, kind="ExternalInput").ap()
    wq = nc.dram_tensor("wq", [D, DL], BF16, kind="ExternalInput").ap()
    wk = nc.dram_tensor("wk", [D, DL], BF16, kind="ExternalInput").ap()
    wv = nc.dram_tensor("wv", [D, DL], BF16, kind="ExternalInput").ap()
    wp = nc.dram_tensor("wp", [DL, D], BF16, kind="ExternalInput").ap()
    out = nc.dram_tensor("out", [T, D], F32, kind="ExternalOutput").ap()
    dbg = {}
    if DEBUG_DUMPS:
        for name, shape, dt in [
            ("dbg_qT", [128, T], BF16), ("dbg_kT", [128, T], BF16),
            ("dbg_v0", [128, KT * 65], BF16),
            ("dbg_v1", [128, KT * 65], BF16),
            ("dbg_e0", [128, QGS], BF16),
            ("dbg_yT", [64, HPD, T], BF16),
            ("dbg_spre", [128, HPD, 16], F32),
            ("dbg_scal", [128, HPD, 16], F32),
        ]:
            dbg[name] = nc.dram_tensor(name, shape, dt,
                                       kind="ExternalOutput").ap()

    with tile.TileContext(nc) as tc:
        with (
            tc.tile_pool(name="const", bufs=1) as cpool,
            tc.tile_pool(name="work", bufs=4) as wpool,
            tc.tile_pool(name="epool", bufs=4) as epool,
            tc.tile_pool(name="opool", bufs=4) as opool,
            tc.tile_pool(name="dram", bufs=1, space="DRAM") as dram_pool,
            tc.tile_pool(name="mm", bufs=2, space="PSUM") as mm_psum,
            tc.tile_pool(name="sc", bufs=2, space="PSUM") as sc_psum,
            tc.tile_pool(name="yp", bufs=2, space="PSUM") as y_psum_pool,
        ):
            # ---- persistent SBUF tensors ----
            xT_sb = cpool.tile([128, 8, T], BF16)       # x^T, d-major
            wq_sb = cpool.tile([128, 8, DL], BF16)
            wk_sb = cpool.tile([128, 8, DL], BF16)
            wv_sb = cpool.tile([128, 8, DL], BF16)
            wp0_sb = cpool.tile([64, D], BF16)          # proj rows of head 0
            wp1_sb = cpool.tile([64, D], BF16)          # proj rows of head 1
            qT_sb = cpool.tile([128, T], BF16)
            kT_sb = cpool.tile([128, T], BF16)
            v0_sb = cpool.tile([128, KT * 65], BF16)    # [t, v|1] head 0
            v1_sb = cpool.tile([128, KT * 65], BF16)
            yT_sb = cpool.tile([64, HPD, T], BF16)      # y^T per head
            spre_sb = cpool.tile([128, HPD, 16], F32)   # softmax sums (q-major)
            scal_sb = cpool.tile([128, HPD, 16], F32)   # 1/sums
            sums_dr = dram_pool.tile([HPD, QG, QGS], F32)  # DRAM bounce buffer

            # ---- input DMAs (split along d so matmuls can start early) ----
            xTr = xT.rearrange("(n p) t -> p n t", p=128)
            wqr = wq.rearrange("(n p) m -> p n m", p=128)
            wkr = wk.rearrange("(n p) m -> p n m", p=128)
            wvr = wv.rearrange("(n p) m -> p n m", p=128)
            for kk in range(8):
                nc.sync.dma_start(xT_sb[:, kk, :], xTr[:, kk, :])
                nc.sync.dma_start(wq_sb[:, kk, :], wqr[:, kk, :])
                nc.sync.dma_start(wk_sb[:, kk, :], wkr[:, kk, :])
                nc.sync.dma_start(wv_sb[:, kk, :], wvr[:, kk, :])
            nc.sync.dma_start(wp0_sb[:], wp[0:64, :])
            nc.sync.dma_start(wp1_sb[:], wp[64:128, :])

            nc.vector.memset(v0_sb[:], 1.0)
            nc.vector.memset(v1_sb[:], 1.0)

            # ---- phase 1: qT / kT = W^T @ x^T  ([128, T] each) ----
            for dst, w_sb in ((qT_sb, wq_sb), (kT_sb, wk_sb)):
                for g in range(QG):
                    ps = mm_psum.tile([128, QGS], F32, tag="mmps")
                    for kk in range(8):
                        nc.tensor.matmul(
                            ps[:], w_sb[:, kk, :],
                            xT_sb[:, kk, g * QGS:(g + 1) * QGS],
                            start=(kk == 0), stop=(kk == 7))
                    nc.vector.tensor_copy(dst[:, g * QGS:(g + 1) * QGS], ps[:])

            # v in natural [t, vdim] layout: lhsT = xT tile (t on out
            # partitions); both heads' halves land in the ones-padded bufs
            for tt in range(KT):
                ps = mm_psum.tile([128, DL], F32, tag="mmps")
                for kk in range(8):
                    nc.tensor.matmul(
                        ps[:], xT_sb[:, kk, tt * 128:(tt + 1) * 128],
                        wv_sb[:, kk, :], start=(kk == 0), stop=(kk == 7))
                nc.vector.tensor_copy(v0_sb[:, tt * 65: tt * 65 + 64],
                                      ps[:, 0:64])
                nc.vector.tensor_copy(v1_sb[:, tt * 65: tt * 65 + 64],
                                      ps[:, 64:128])

            # ---- phase 2: attention per q-group ----
            for g in range(QG):
                qsl = slice(g * QGS, (g + 1) * QGS)
                y0 = y_psum_pool.tile([65, QGS], F32, tag="y")
                y1 = y_psum_pool.tile([65, QGS], F32, tag="y")
                for kk in range(KT):
                    ksl = slice(kk * 128, (kk + 1) * 128)
                    # both heads' scores for this k-tile in ONE 2-bank tile:
                    # the row-tiled pair gates on the same release, stays
                    # adjacent in the PE stream, and overlaps (rows 0-63 vs
                    # 64-127 of the array)
                    sc = sc_psum.tile([128, 2 * QGS], F32, tag="sc")
                    nc.tensor.matmul(sc[:, 0:QGS], kT_sb[0:64, ksl],
                                     qT_sb[0:64, qsl], start=True,
                                     stop=True, tile_position=(0, 0))
                    nc.tensor.matmul(sc[:, QGS:2 * QGS], kT_sb[64:128, ksl],
                                     qT_sb[64:128, qsl], start=True,
                                     stop=True, tile_position=(64, 0))
                    # one exp covers both heads
                    e = epool.tile([128, 2 * QGS], BF16, tag="e")
                    nc.scalar.activation(e[:], sc[:],
                                         mybir.ActivationFunctionType.Exp)
                    if DEBUG_DUMPS and g == 0 and kk == 0:
                        nc.sync.dma_start(dbg["dbg_e0"][:], e[:, 0:QGS])
                    nc.tensor.matmul(y0[:], v0_sb[:, kk * 65: kk * 65 + 65],
                                     e[:, 0:QGS], start=(kk == 0),
                                     stop=(kk == KT - 1))
                    nc.tensor.matmul(y1[:], v1_sb[:, kk * 65: kk * 65 + 65],
                                     e[:, QGS:2 * QGS], start=(kk == 0),
                                     stop=(kk == KT - 1))
                # y rows 0-63 -> bf16 yT; row 64 (sums) -> fp32, transposed
                nc.vector.tensor_copy(yT_sb[:, 0, qsl], y0[0:64, :])
                nc.vector.tensor_copy(yT_sb[:, 1, qsl], y1[0:64, :])
                srow0 = wpool.tile([65, QGS], F32, tag="srow")
                srow1 = wpool.tile([65, QGS], F32, tag="srow")
                nc.vector.tensor_copy(srow0[64:65, :], y0[64:65, :])
                nc.vector.tensor_copy(srow1[64:65, :], y1[64:65, :])
                # transpose sums [1, 512] -> [128, 4] via a DRAM bounce
                # (SBUF APs can't move free elements onto partitions; DRAM
                # APs are linear so the scatter is legal on the read-back)
                nc.sync.dma_start(sums_dr[0, g, :], srow0[64:65, :])
                nc.sync.dma_start(sums_dr[1, g, :], srow1[64:65, :])
                for h in range(HPD):
                    nc.sync.dma_start(
                        spre_sb[:, h, g * 4:(g + 1) * 4],
                        sums_dr[h, g, :].rearrange("(c p) -> p c", c=4, p=128))
                nc.vector.reciprocal(scal_sb[:, :, g * 4:(g + 1) * 4],
                                     spre_sb[:, :, g * 4:(g + 1) * 4])

                # proj for this q-group (overlaps the next group's attention)
                for qt in range(g * 4, (g + 1) * 4):
                    tsl = slice(qt * 128, (qt + 1) * 128)
                    for ngi in range(NG):
                        nsl = slice(ngi * 512, (ngi + 1) * 512)
                        pj0 = mm_psum.tile([128, 512], F32, tag="mmps")
                        pj1 = mm_psum.tile([128, 512], F32, tag="mmps")
                        nc.tensor.matmul(pj0[:], yT_sb[:, 0, tsl],
                                         wp0_sb[:, nsl], start=True, stop=True)
                        nc.tensor.matmul(pj1[:], yT_sb[:, 1, tsl],
                                         wp1_sb[:, nsl], start=True, stop=True)
                        acc = opool.tile([128, 512], F32, tag="acc")
                        o = opool.tile([128, 512], F32, tag="o")
                        if g == QG - 1:
                            # tail: ACT is idle once the exps are done — use
                            # it for the h0 scale so DVE isn't the critical
                            # path of the epilogue
                            nc.scalar.mul(acc[:], pj0[:],
                                          scal_sb[:, 0, qt:qt + 1])
                        else:
                            nc.vector.tensor_scalar_mul(
                                acc[:], pj0[:], scal_sb[:, 0, qt:qt + 1])
                        nc.vector.scalar_tensor_tensor(
                            o[:], pj1[:], scal_sb[:, 1, qt:qt + 1], acc[:],
                            op0=mybir.AluOpType.mult, op1=mybir.AluOpType.add)
                        nc.gpsimd.dma_start(out[tsl, nsl], o[:])

            if DEBUG_DUMPS:
                nc.sync.dma_start(dbg["dbg_qT"][:], qT_sb[:])
                nc.sync.dma_start(dbg["dbg_kT"][:], kT_sb[:])
                nc.sync.dma_start(dbg["dbg_v0"][:], v0_sb[:])
                nc.sync.dma_start(dbg["dbg_v1"][:], v1_sb[:])
                nc.sync.dma_start(dbg["dbg_yT"][:], yT_sb[:])
                nc.sync.dma_start(dbg["dbg_spre"][:], spre_sb[:])
                nc.sync.dma_start(dbg["dbg_scal"][:], scal_sb[:])

    nc.compile()
    return nc


_nc_cache = None


def kernel(x: np.ndarray, W_qkv: np.ndarray, W_proj: np.ndarray) -> np.ndarray:
    global _nc_cache, last_results
    assert x.shape == (B, T, D)
    x2d = np.ascontiguousarray(x.reshape(T, D))
    xT = np.ascontiguousarray(x2d.T).astype(ml_dtypes.bfloat16)
    scale = 1.0 / np.sqrt(np.float32(HD))

    in_maps = []
    for dev in range(NCORES):
        cs = slice(dev * DL, (dev + 1) * DL)
        wq = (W_qkv[:, cs.start:cs.stop] * scale).astype(ml_dtypes.bfloat16)
        wk = W_qkv[:, D + dev * DL: D + (dev + 1) * DL].astype(ml_dtypes.bfloat16)
        wv = W_qkv[:, 2 * D + dev * DL: 2 * D + (dev + 1) * DL].astype(
            ml_dtypes.bfloat16)
        wp = W_proj[dev * DL:(dev + 1) * DL, :].astype(ml_dtypes.bfloat16)
        in_maps.append({
            "xT": xT,
            "wq": np.ascontiguousarray(wq),
            "wk": np.ascontiguousarray(wk),
            "wv": np.ascontiguousarray(wv),
            "wp": np.ascontiguousarray(wp),
        })

    if _nc_cache is None:
        _nc_cache = build_nc()
    res = bass_utils.run_bass_kernel_spmd(
        _nc_cache, in_maps, core_ids=list(range(NCORES)),
        tmpdir=os.environ.get("BASS_KERNEL_TMPDIR"))
    last_results = res
    total = np.zeros((T, D), dtype=np.float64)
    for dev in range(NCORES):
        total += res.results[dev]["out"].astype(np.float64)
    return total.astype(np.float32).reshape(B, T, D)


if __name__ == "__main__":
    rng = np.random.default_rng(0)
    x = rng.standard_normal((B, T, D)).astype(np.float32)
    wqkv = (rng.standard_normal((D, 3 * D)) * 0.02).astype(np.float32)
    wproj = (rng.standard_normal((D, D)) * 0.02).astype(np.float32)
    y = kernel(x, wqkv, wproj)
    print("kernel output", y.shape, y.dtype, float(np.abs(y).mean()))


# revision 20
# speedup vs baseline: 1.0442x; 1.0165x over previous
"""Trainium2 Bass kernel for 16-head self-attention (B=1, T=2048, d=1024).

Sharding: 2 heads per NeuronCore (tensor-parallel over QKV columns / proj
rows) across 8 cores; each core emits a partial [T, d] projection output,
summed on the host.

Device-side layout (per core, heads h0/h1 stacked on SBUF partitions):
  qT/kT  [128, T]   head-dim-major (h0 dims on partitions 0-63, h1 on 64-127)
  scores ST[k, q] via row-tiled matmul pairs (both heads concurrent on PE)
  exp on ACT (no max subtraction needed: logits are O(1) by construction)
  y^T = [v | 1]^T @ exp(ST)  -> row 64 of the PSUM tile is the softmax sum
  per-head proj + per-partition renormalization (sums transposed via DMA)
"""
import sys, os, types

sys.path.insert(0, "/opt/trn_rl_repo")

import numpy as np
import ml_dtypes

import concourse.bass as bass
import concourse.bacc as bacc
import concourse.tile as tile
from concourse import mybir
from concourse import bass_utils

BF16 = mybir.dt.bfloat16
F32 = mybir.dt.float32

B, T, D = 1, 2048, 1024
H = 16
HD = D // H          # 64
NCORES = 8
HPD = H // NCORES    # 2 heads per device
DL = HPD * HD        # 128 local head dims per device
QG = 4               # q groups of 512
QGS = T // QG        # 512
KT = T // 128        # 16 k tiles
NG = D // 512        # 2 output column groups

last_results = None  # BassKernelResults of the most recent run (for test.py)


DEBUG_DUMPS = os.environ.get("KERNEL_DEBUG_DUMPS", "0") == "1"


def build_nc():
    nc = bacc.Bacc("TRN2", target_bir_lowering=False, debug=False,
                   num_devices=NCORES)
    xT = nc.dram_tensor("xT", [D, T], BF16, kind="ExternalInput").ap()
    wq = nc.dram_tensor("wq", [D, DL], BF16, kind="ExternalInput").ap()
    wk = nc.dram_tensor("wk", [D, DL], BF16, kind="ExternalInput").ap()
    wv = nc.dram_tensor("wv", [D, DL], BF16, kind="ExternalInput").ap()
    wp = nc.dram_tensor("wp", [DL, D], BF16, kind="ExternalInput").ap()
    out = nc.dram_tensor("out", [T, D], F32, kind="ExternalOutput").ap()
    dbg = {}
    if DEBUG_DUMPS:
        for name, shape, dt in [
            ("dbg_qT", [128, T], BF16), ("dbg_kT", [128, T], BF16),
            ("dbg_v0", [128, KT * 65], BF16),
            ("dbg_v1", [128, KT * 65], BF16),
            ("dbg_e0", [128, QGS], BF16),
            ("dbg_yT", [64, HPD, T], BF16),
            ("dbg_spre", [128, HPD, 16], F32),
            ("dbg_scal", [128, HPD, 16], F32),
        ]:
            dbg[name] = nc.dram_tensor(name, shape, dt,
                                       kind="ExternalOutput").ap()

    with tile.TileContext(nc) as tc:
        with (
            tc.tile_pool(name="const", bufs=1) as cpool,
            tc.tile_pool(name="work", bufs=4) as wpool,
            tc.tile_pool(name="epool", bufs=4) as epool,
            tc.tile_pool(name="opool", bufs=4) as opool,
            tc.tile_pool(name="dram", bufs=1, space="DRAM") as dram_pool,
            tc.tile_pool(name="mm", bufs=2, space="PSUM") as mm_psum,
            tc.tile_pool(name="sc", bufs=2, space="PSUM") as sc_psum,
            tc.tile_pool(name="yp", bufs=2, space="PSUM") as y_psum_pool,
        ):
            # ---- persistent SBUF tensors ----
            xT_sb = cpool.tile([128, 8, T], BF16)       # x^T, d-major
            wq_sb = cpool.tile([128, 8, DL], BF16)
            wk_sb = cpool.tile([128, 8, DL], BF16)
            wv_sb = cpool.tile([128, 8, DL], BF16)
            wp0_sb = cpool.tile([64, D], BF16)          # proj rows of head 0
            wp1_sb = cpool.tile([64, D], BF16)          # proj rows of head 1
            qT_sb = cpool.tile([128, T], BF16)
            kT_sb = cpool.tile([128, T], BF16)
            v0_sb = cpool.tile([128, KT * 65], BF16)    # [t, v|1] head 0
            v1_sb = cpool.tile([128, KT * 65], BF16)
            yT_sb = cpool.tile([64, HPD, T], BF16)      # y^T per head
            spre_sb = cpool.tile([128, HPD, 16], F32)   # softmax sums (q-major)
            scal_sb = cpool.tile([128, HPD, 16], F32)   # 1/sums
            sums_dr = dram_pool.tile([HPD, QG, QGS], F32)  # DRAM bounce buffer

            # ---- input DMAs (split along d so matmuls can start early) ----
            xTr = xT.rearrange("(n p) t -> p n t", p=128)
            wqr = wq.rearrange("(n p) m -> p n m", p=128)
            wkr = wk.rearrange("(n p) m -> p n m", p=128)
            wvr = wv.rearrange("(n p) m -> p n m", p=128)
            for kk in range(8):
                eng = nc.sync if kk % 2 == 0 else nc.gpsimd
                eng.dma_start(xT_sb[:, kk, :], xTr[:, kk, :])
                eng.dma_start(wq_sb[:, kk, :], wqr[:, kk, :])
                eng.dma_start(wk_sb[:, kk, :], wkr[:, kk, :])
                eng.dma_start(wv_sb[:, kk, :], wvr[:, kk, :])
            nc.sync.dma_start(wp0_sb[:], wp[0:64, :])
            nc.gpsimd.dma_start(wp1_sb[:], wp[64:128, :])

            nc.vector.memset(v0_sb[:], 1.0)
            nc.vector.memset(v1_sb[:], 1.0)

            # ---- phase 1: qT / kT = W^T @ x^T  ([128, T] each) ----
            for dst, w_sb in ((qT_sb, wq_sb), (kT_sb, wk_sb)):
                for g in range(QG):
                    ps = mm_psum.tile([128, QGS], F32, tag="mmps")
                    for kk in range(8):
                        nc.tensor.matmul(
                            ps[:], w_sb[:, kk, :],
                            xT_sb[:, kk, g * QGS:(g + 1) * QGS],
                            start=(kk == 0), stop=(kk == 7))
                    nc.vector.tensor_copy(dst[:, g * QGS:(g + 1) * QGS], ps[:])

            # v in natural [t, vdim] layout: lhsT = xT tile (t on out
            # partitions); both heads' halves land in the ones-padded bufs
            for tt in range(KT):
                ps = mm_psum.tile([128, DL], F32, tag="mmps")
                for kk in range(8):
                    nc.tensor.matmul(
                        ps[:], xT_sb[:, kk, tt * 128:(tt + 1) * 128],
                        wv_sb[:, kk, :], start=(kk == 0), stop=(kk == 7))
                nc.vector.tensor_copy(v0_sb[:, tt * 65: tt * 65 + 64],
                                      ps[:, 0:64])
                nc.vector.tensor_copy(v1_sb[:, tt * 65: tt * 65 + 64],
                                      ps[:, 64:128])

            # ---- phase 2: attention per q-group ----
            for g in range(QG):
                qsl = slice(g * QGS, (g + 1) * QGS)
                y0 = y_psum_pool.tile([65, QGS], F32, tag="y")
                y1 = y_psum_pool.tile([65, QGS], F32, tag="y")
                for kk in range(KT):
                    ksl = slice(kk * 128, (kk + 1) * 128)
                    # both heads' scores for this k-tile in ONE 2-bank tile:
                    # the row-tiled pair gates on the same release, stays
                    # adjacent in the PE stream, and overlaps (rows 0-63 vs
                    # 64-127 of the array)
                    sc = sc_psum.tile([128, 2 * QGS], F32, tag="sc")
                    nc.tensor.matmul(sc[:, 0:QGS], kT_sb[0:64, ksl],
                                     qT_sb[0:64, qsl], start=True,
                                     stop=True, tile_position=(0, 0))
                    nc.tensor.matmul(sc[:, QGS:2 * QGS], kT_sb[64:128, ksl],
                                     qT_sb[64:128, qsl], start=True,
                                     stop=True, tile_position=(64, 0))
                    # one exp covers both heads
                    e = epool.tile([128, 2 * QGS], BF16, tag="e")
                    nc.scalar.activation(e[:], sc[:],
                                         mybir.ActivationFunctionType.Exp)
                    if DEBUG_DUMPS and g == 0 and kk == 0:
                        nc.sync.dma_start(dbg["dbg_e0"][:], e[:, 0:QGS])
                    nc.tensor.matmul(y0[:], v0_sb[:, kk * 65: kk * 65 + 65],
                                     e[:, 0:QGS], start=(kk == 0),
                                     stop=(kk == KT - 1))
                    nc.tensor.matmul(y1[:], v1_sb[:, kk * 65: kk * 65 + 65],
                                     e[:, QGS:2 * QGS], start=(kk == 0),
                                     stop=(kk == KT - 1))
                # y rows 0-63 -> bf16 yT; row 64 (sums) -> fp32, transposed
                nc.vector.tensor_copy(yT_sb[:, 0, qsl], y0[0:64, :])
                nc.vector.tensor_copy(yT_sb[:, 1, qsl], y1[0:64, :])
                srow0 = wpool.tile([65, QGS], F32, tag="srow")
                srow1 = wpool.tile([65, QGS], F32, tag="srow")
                nc.vector.tensor_copy(srow0[64:65, :], y0[64:65, :])
                nc.vector.tensor_copy(srow1[64:65, :], y1[64:65, :])
                # transpose sums [1, 512] -> [128, 4] via a DRAM bounce
                # (SBUF APs can't move free elements onto partitions; DRAM
                # APs are linear so the scatter is legal on the read-back)
                nc.sync.dma_start(sums_dr[0, g, :], srow0[64:65, :])
                nc.gpsimd.dma_start(sums_dr[1, g, :], srow1[64:65, :])
                for h, eng in ((0, nc.sync), (1, nc.gpsimd)):
                    eng.dma_start(
                        spre_sb[:, h, g * 4:(g + 1) * 4],
                        sums_dr[h, g, :].rearrange("(c p) -> p c", c=4, p=128))
                nc.vector.reciprocal(scal_sb[:, :, g * 4:(g + 1) * 4],
                                     spre_sb[:, :, g * 4:(g + 1) * 4])

                # proj for this q-group (overlaps the next group's attention)
                for qt in range(g * 4, (g + 1) * 4):
                    tsl = slice(qt * 128, (qt + 1) * 128)
                    for ngi in range(NG):
                        nsl = slice(ngi * 512, (ngi + 1) * 512)
                        pj0 = mm_psum.tile([128, 512], F32, tag="mmps")
                        pj1 = mm_psum.tile([128, 512], F32, tag="mmps")
                        nc.tensor.matmul(pj0[:], yT_sb[:, 0, tsl],
                                         wp0_sb[:, nsl], start=True, stop=True)
                        nc.tensor.matmul(pj1[:], yT_sb[:, 1, tsl],
                                         wp1_sb[:, nsl], start=True, stop=True)
                        acc = opool.tile([128, 512], F32, tag="acc")
                        o = opool.tile([128, 512], F32, tag="o")
                        if g == QG - 1:
                            # tail: ACT is idle once the exps are done — use
                            # it for the h0 scale so DVE isn't the critical
                            # path of the epilogue
                            nc.scalar.mul(acc[:], pj0[:],
                                          scal_sb[:, 0, qt:qt + 1])
                        else:
                            nc.vector.tensor_scalar_mul(
                                acc[:], pj0[:], scal_sb[:, 0, qt:qt + 1])
                        nc.vector.scalar_tensor_tensor(
                            o[:], pj1[:], scal_sb[:, 1, qt:qt + 1], acc[:],
                            op0=mybir.AluOpType.mult, op1=mybir.AluOpType.add)
                        nc.gpsimd.dma_start(out[tsl, nsl], o[:])

            if DEBUG_DUMPS:
                nc.sync.dma_start(dbg["dbg_qT"][:], qT_sb[:])
                nc.sync.dma_start(dbg["dbg_kT"][:], kT_sb[:])
                nc.sync.dma_start(dbg["dbg_v0"][:], v0_sb[:])
                nc.sync.dma_start(dbg["dbg_v1"][:], v1_sb[:])
                nc.sync.dma_start(dbg["dbg_yT"][:], yT_sb[:])
                nc.sync.dma_start(dbg["dbg_spre"][:], spre_sb[:])
                nc.sync.dma_start(dbg["dbg_scal"][:], scal_sb[:])

    nc.compile()
    return nc


_nc_cache = None


def kernel(x: np.ndarray, W_qkv: np.ndarray, W_proj: np.ndarray) -> np.ndarray:
    global _nc_cache, last_results
    assert x.shape == (B, T, D)
    x2d = np.ascontiguousarray(x.reshape(T, D))
    xT = np.ascontiguousarray(x2d.T).astype(ml_dtypes.bfloat16)
    scale = 1.0 / np.sqrt(np.float32(HD))

    in_maps = []
    for dev in range(NCORES):
        cs = slice(dev * DL, (dev + 1) * DL)
        wq = (W_qkv[:, cs.start:cs.stop] * scale).astype(ml_dtypes.bfloat16)
        wk = W_qkv[:, D + dev * DL: D + (dev + 1) * DL].astype(ml_dtypes.bfloat16)
        wv = W_qkv[:, 2 * D + dev * DL: 2 * D + (dev + 1) * DL].astype(
            ml_dtypes.bfloat16)
        wp = W_proj[dev * DL:(dev + 1) * DL, :].astype(ml_dtypes.bfloat16)
        in_maps.append({
            "xT": xT,
            "wq": np.ascontiguousarray(wq),
            "wk": np.ascontiguousarray(wk),
            "wv": np.ascontiguousarray(wv),
            "wp": np.ascontiguousarray(wp),
        })

    if _nc_cache is None:
        _nc_cache = build_nc()
    res = bass_utils.run_bass_kernel_spmd(
        _nc_cache, in_maps, core_ids=list(range(NCORES)),
        tmpdir=os.environ.get("BASS_KERNEL_TMPDIR"))
    last_results = res
    total = np.zeros((T, D), dtype=np.float64)
    for dev in range(NCORES):
        total += res.results[dev]["out"].astype(np.float64)
    return total.astype(np.float32).reshape(B, T, D)


if __name__ == "__main__":
    rng = np.random.default_rng(0)
    x = rng.standard_normal((B, T, D)).astype(np.float32)
    wqkv = (rng.standard_normal((D, 3 * D)) * 0.02).astype(np.float32)
    wproj = (rng.standard_normal((D, D)) * 0.02).astype(np.float32)
    y = kernel(x, wqkv, wproj)
    print("kernel output", y.shape, y.dtype, float(np.abs(y).mean()))
